# revision 35
# baseline (speedup 1.0000x reference)
"""Fused OT-DTW l2 cost-matrix kernel for Trainium2 (8 NeuronCores, SPMD).

mat_cost[i,j] = sum_{t,p,d} pi[cl(i)][t,p] * (X[i,t,d] - Y[j,p,d])^2
             = C1[i] + C2[cl(i), j] - 2 * C3[i,j]

with C3[i,j] = sum_{p,d} XP[i,p,d] * Y[j,p,d],  XP[i] = pi[cl(i)].T @ X[i].

pi is a monotone DTW path matrix: every column p holds a contiguous run of
ones [lo_p, hi_p]. So XP[i,p,:] = Xc[i,hi_p,:] - Xc[i,lo_p-1,:] with Xc the
prefix sum of X over t — the host computes XP exactly in fp32 in O(NX*TP*D)
and quantizes to fp8 (structure is validated; falls back to exact per-class
BLAS if pi is not a 0/1 path matrix). The device runs only the heavy
contraction C3 (137 GFLOP total) as fp8 DoubleRow matmuls. The tiny rank-1
corrections C1/C2 (<0.2% of FLOPs) are applied on the host in fp32.

Sharding splits the CONTRACTION: core k takes d in [16k, 16k+16) of both
XP and Y for ALL rows/cols, computes the full [1024,1024] partial C3 over
its d-slice, and the host sums the 8 fp32 partials. That makes both
operands SBUF-resident (8.4MB each; 20.8MB total DMA/core vs a ~116us PE
floor), so the kernel is cleanly PE-bound. To keep the full 128-partition
contraction with only 16 d's, the host packs 8 consecutive p's x 16 d's
per partition: part = 16*(p%8) + d_local, q = p//8. DoubleRow pairs run
over (q, q+1), i.e. K = 16 p's x 16 d's = 256 per matmul.

Device layouts (host pre-packs; every DMA contiguous):
  xq [part, q, i]  fp8, packed XP  (64KB/partition, resident)
  yq [part, q, j]  fp8, packed Y   (64KB/partition, resident)
Loop: 2 col-halves x 32 q-pairs x 8 row-tiles of DR matmuls (512 total,
N=512), accumulating 8 PSUM banks per col-half; rhs is shared across the
8 row-tiles of each (c, q-pair). Output c3 [1024, 1024] bf16 per core
(abs-error budget is ~35x larger than the bf16 rounding it adds), drained
to DRAM staggered so only the last tile's copy+DMA trails the final
matmul. A scratch-matmul burst at engine boot warms the PE HAM clock-gate
to 8/8 while the first operand chunks are still in flight.
"""

import os
import sys
import types

import numpy as np
import ml_dtypes

NX, NY, T, TP, D, C = 1024, 1024, 512, 512, 128, 8
N_CORES = 8
DL = D // N_CORES           # 16 d's per core
PP = 128 // DL              # 8 p's packed per partition column
NQ = TP // PP               # 64 q's
F8 = ml_dtypes.float8_e4m3fn


def _ensure_axon_hooks():
    """concourse.bass_utils imports antenv.axon_hooks when tracing under
    axon; some images lack that submodule. Provide it, and register the
    NTFF profile hook if the boot path didn't."""
    try:
        import antenv
    except ImportError:
        return
    try:
        from antenv import axon_hooks  # noqa: F401
    except ImportError:
        mod = types.ModuleType("antenv.axon_hooks")
        mod._hook = None

        def _set(h):
            mod._hook = h

        def _get():
            return mod._hook

        mod.set_axon_ntff_profile_hook = _set
        mod.get_axon_ntff_profile_hook = _get
        sys.modules["antenv.axon_hooks"] = mod
        antenv.axon_hooks = mod
    from antenv.axon_hooks import (
        get_axon_ntff_profile_hook,
        set_axon_ntff_profile_hook,
    )

    if get_axon_ntff_profile_hook() is None:
        try:
            from trn_agent_boot.trn_boot import _ntff_profile_via_ctypes

            hook = _ntff_profile_via_ctypes("/opt/axon/libaxon_pjrt.so")
            if hook is not None:
                set_axon_ntff_profile_hook(hook)
        except Exception:
            pass


_ensure_axon_hooks()

import concourse.bass as bass  # noqa: E402, F401
import concourse.tile as tile  # noqa: E402
from concourse import bacc, mybir  # noqa: E402
from concourse.bass_utils import run_bass_kernel_spmd  # noqa: E402

_PROGRAM_CACHE = {}
LAST_RUN = None  # BassKernelResults of the most recent kernel() call


def _build_program():
    if "nc" in _PROGRAM_CACHE:
        return _PROGRAM_CACHE["nc"]
    f8 = mybir.dt.float8e4
    f32 = mybir.dt.float32
    DR = mybir.MatmulPerfMode.DoubleRow
    nc = bacc.Bacc("TRN2", target_bir_lowering=False, debug=False,
                   num_devices=N_CORES)
    bf16 = mybir.dt.bfloat16
    xq_d = nc.dram_tensor("xq", [128, NQ, NX], f8, kind="ExternalInput").ap()
    yq_d = nc.dram_tensor("yq", [128, NQ, NY], f8, kind="ExternalInput").ap()
    c3 = nc.dram_tensor("c3", [NX, NY], bf16, kind="ExternalOutput").ap()

    with tile.TileContext(nc) as tc:
        with (
            tc.tile_pool(name="ops", bufs=1) as op_pool,
            tc.tile_pool(name="outsb", bufs=4) as out_pool,
        ):
            xq = op_pool.tile([128, NQ, NX], f8)
            yq = op_pool.tile([128, NQ, NY], f8)

            # PE warmup: ~3.4us of scratch matmuls starting at engine boot
            # (~7.3us), i.e. while no operand data exists yet, flip the HAM
            # clock-gate to 8/8 right as the first chunks land, so all real
            # matmuls run warm (values never read).
            wsrc = op_pool.tile([128, 128], f8)
            wdst = op_pool.tile([1, 8], f32)
            nc.vector.memset(wsrc[:], 0.0)
            with tc.tile_pool(name="warmps", bufs=1,
                              space="PSUM") as warmps_pool:
                wacc = warmps_pool.tile([128, 128], f32)
                for w in range(32):
                    nc.tensor.matmul(wacc[:], wsrc[:], wsrc[:],
                                     start=True, stop=True)

            # Operand loads in q-pair chunks aligned to s-blocks: matmul
            # block s of col-half 0 waits on exactly one xq chunk + one
            # yq cols[0:512] chunk. yq cols[512:1024] (not needed until
            # col-half 1 at ~66us) trails a few chunks behind, halving
            # early supply pressure so DMA stays ahead of consumption
            # from the first block on.
            QC = 2
            NG = NQ // QC
            for g in range(NG):
                a, b = g * QC, (g + 1) * QC
                nc.sync.dma_start(xq[:, a:b, :], xq_d[:, a:b, :])
                nc.sync.dma_start(yq[:, a:b, 0:512], yq_d[:, a:b, 0:512])
                if g >= 8:
                    a2, b2 = (g - 8) * QC, (g - 7) * QC
                    nc.sync.dma_start(yq[:, a2:b2, 512:1024],
                                      yq_d[:, a2:b2, 512:1024])
            for g in range(NG - 8, NG):
                a, b = g * QC, (g + 1) * QC
                nc.sync.dma_start(yq[:, a:b, 512:1024],
                                  yq_d[:, a:b, 512:1024])
            # One tiny ACT op so the lazy activation-table load (~1.3us)
            # runs at boot, long before the first PSUM drain needs the
            # Scalar engine at ~62us.
            nc.scalar.copy(wdst[:], wsrc[0:1, 0:8])

            # Partial C3 over this core's d-slice: 2 col-halves x 32 q-pairs
            # x 8 row-tiles; 8 PSUM banks accumulate one col-half's row
            # tiles, then drain to SBUF/DRAM while the other half runs.
            # Col-half 0 iterates q-outer (chunk-streaming friendly);
            # col-half 1 (operands fully resident by then) iterates
            # row-tile-outer so the 8 tiles finish staggered and their
            # copies + output DMAs overlap compute instead of draining
            # serially after the last matmul.
            with tc.tile_pool(name="psB", bufs=1, space="PSUM") as psB_pool:
                def drain(rt, c, acc, split=False):
                    out = out_pool.tile([128, 512], bf16, name="out",
                                        tag="out")
                    h = 256
                    nc.vector.tensor_copy(out[:, 0:h], acc[:, 0:h])
                    nc.scalar.copy(out[:, h:512], acc[:, h:512])
                    dst = c3[rt * 128:(rt + 1) * 128,
                             c * 512:(c + 1) * 512]
                    if split:
                        # Last tile: two half-DMAs so each half departs as
                        # its copy lands and the ~2us completion receipts
                        # overlap instead of chaining.
                        nc.sync.dma_start(dst[:, 0:h], out[:, 0:h])
                        nc.sync.dma_start(dst[:, h:512], out[:, h:512])
                    else:
                        nc.sync.dma_start(dst, out[:])

                accs = [psB_pool.tile([128, 512], f32, name=f"acc{rt}",
                                      tag=f"acc{rt}")
                        for rt in range(8)]
                for s in range(NQ // 2):
                    q = 2 * s
                    st, sp = (s == 0), (s == NQ // 2 - 1)
                    rhs = yq[:, q:q + 2, 0:512]
                    for rt in range(8):
                        nc.tensor.matmul(
                            accs[rt][:],
                            xq[:, q:q + 2, rt * 128:(rt + 1) * 128],
                            rhs, start=st, stop=sp, perf_mode=DR)
                for rt in range(8):
                    drain(rt, 0, accs[rt])

                for rt in range(7):
                    acc = psB_pool.tile([128, 512], f32, name=f"acc{rt}",
                                        tag=f"acc{rt}")
                    for s in range(NQ // 2):
                        q = 2 * s
                        st, sp = (s == 0), (s == NQ // 2 - 1)
                        nc.tensor.matmul(
                            acc[:], xq[:, q:q + 2, rt * 128:(rt + 1) * 128],
                            yq[:, q:q + 2, 512:1024],
                            start=st, stop=sp, perf_mode=DR)
                    drain(rt, 1, acc)

                # Last tile: two N=256 column groups in one PSUM bank so
                # group A's copy + DMA + ~2us receipt overlap group B's
                # matmuls, and only a [128,256] drain trails the final
                # matmul.
                acc = psB_pool.tile([128, 512], f32, name="acc7",
                                    tag="acc7")
                lhsT7 = xq[:, 0:NQ, 7 * 128:8 * 128]
                for grp in range(2):
                    jc = 512 + 256 * grp
                    for s in range(NQ // 2):
                        q = 2 * s
                        st, sp = (s == 0), (s == NQ // 2 - 1)
                        nc.tensor.matmul(
                            acc[:, 256 * grp:256 * (grp + 1)],
                            xq[:, q:q + 2, 7 * 128:8 * 128],
                            yq[:, q:q + 2, jc:jc + 256],
                            start=st, stop=sp, perf_mode=DR)
                    out = out_pool.tile([128, 256], bf16, name="out7",
                                        tag="out7")
                    h = 128
                    nc.vector.tensor_copy(
                        out[:, 0:h], acc[:, 256 * grp:256 * grp + h])
                    nc.scalar.copy(
                        out[:, h:256], acc[:, 256 * grp + h:256 * (grp + 1)])
                    nc.sync.dma_start(
                        c3[896:1024, jc:jc + 256], out[:])

    nc.compile()
    _PROGRAM_CACHE["nc"] = nc
    return nc


def _host_xp(X, pi, classe):
    """XP[i, p, d] = sum_t pi[cl(i), t, p] * X[i, t, d], exact in fp32.

    Fast path uses the DTW-path structure (each pi column = contiguous run
    of ones): XP rows are differences of two prefix sums of X over t.
    Falls back to per-class BLAS if pi is not a 0/1 path matrix.
    """
    lo = np.empty((C, TP), np.int64)
    hi = np.empty((C, TP), np.int64)
    pib = pi != 0
    ok = bool(np.isin(pi, (0.0, 1.0)).all())
    if ok:
        for c in range(C):
            m = pib[c]
            cnt = m.sum(0)
            lo_c = m.argmax(0)
            hi_c = T - 1 - m[::-1].argmax(0)
            if not ((cnt > 0).all() and ((hi_c - lo_c + 1) == cnt).all()):
                ok = False
                break
            lo[c], hi[c] = lo_c, hi_c
    if ok:
        Xcp = np.zeros((NX, T + 1, D), np.float32)
        np.cumsum(X, axis=1, out=Xcp[:, 1:])
        ar = np.arange(NX)[:, None]
        XP = Xcp[ar, hi[classe] + 1] - Xcp[ar, lo[classe]]
    else:
        XP = np.empty((NX, TP, D), np.float32)
        for c in range(C):
            rows = np.nonzero(classe == c)[0]
            if rows.size:
                XP[rows] = np.einsum("ntd,tp->npd", X[rows], pi[c],
                                     optimize=True)
    return XP


def _pack(A8):
    """[n, TP, D] fp8 -> T[pp, d, q, n] contiguous, so core k's operand is
    T[:, DL*k:DL*(k+1)].reshape(128, NQ, n) with part = PP*(p%PP)+d_local
    ... i.e. part = DL*(p%PP) + d_local, q = p//PP."""
    n = A8.shape[0]
    T_ = A8.reshape(n, NQ, PP, D).transpose(2, 3, 1, 0)   # [pp, d, q, n]
    return np.ascontiguousarray(T_)


def kernel(X, Y, pi, classe):
    global LAST_RUN
    assert X.shape == (NX, T, D) and Y.shape == (NY, TP, D)
    assert pi.shape == (C, T, TP) and classe.shape == (NX,)
    X = np.asarray(X, dtype=np.float32)
    Y = np.asarray(Y, dtype=np.float32)
    pi = np.asarray(pi, dtype=np.float32)
    classe = np.asarray(classe)

    nc = _build_program()

    XP = _host_xp(X, pi, classe)                       # [NX, TP, D] fp32
    xq_t = _pack(XP.astype(F8))                        # [PP, D, NQ, NX]
    yq_t = _pack(Y.astype(F8))                         # [PP, D, NQ, NY]
    in_maps = []
    for k in range(N_CORES):
        ds = slice(DL * k, DL * (k + 1))
        in_maps.append({
            "xq": np.ascontiguousarray(xq_t[:, ds]).reshape(128, NQ, NX),
            "yq": np.ascontiguousarray(yq_t[:, ds]).reshape(128, NQ, NY),
        })

    trace = bool(os.environ.get("BASS_TRACE"))
    LAST_RUN = run_bass_kernel_spmd(nc, in_maps, list(range(N_CORES)),
                                    trace=trace)
    C3 = LAST_RUN.results[0]["c3"].astype(np.float32)
    for k in range(1, N_CORES):
        C3 += LAST_RUN.results[k]["c3"].astype(np.float32)

    # Host epilogue: rank-1 corrections (0.15% of FLOPs).
    row_c = pi.sum(-1)                                 # [C, T]
    col_c = pi.sum(1)                                  # [C, TP]
    SX = np.einsum("itd,itd->it", X, X)                # [NX, T]
    SY = np.einsum("jpd,jpd->jp", Y, Y)                # [NY, TP]
    C1 = np.einsum("it,it->i", SX, row_c[classe])      # [NX]
    C2 = col_c @ SY.T                                  # [C, NY]
    return (C1[:, None] + C2[classe] - 2.0 * C3).astype(np.float32)


# revision 36
# speedup vs baseline: 1.0282x; 1.0282x over previous
"""Fused OT-DTW l2 cost-matrix kernel for Trainium2 (8 NeuronCores, SPMD).

mat_cost[i,j] = sum_{t,p,d} pi[cl(i)][t,p] * (X[i,t,d] - Y[j,p,d])^2
             = C1[i] + C2[cl(i), j] - 2 * C3[i,j]

with C3[i,j] = sum_{p,d} XP[i,p,d] * Y[j,p,d],  XP[i] = pi[cl(i)].T @ X[i].

pi is a monotone DTW path matrix: every column p holds a contiguous run of
ones [lo_p, hi_p]. So XP[i,p,:] = Xc[i,hi_p,:] - Xc[i,lo_p-1,:] with Xc the
prefix sum of X over t — the host computes XP exactly in fp32 in O(NX*TP*D)
and quantizes to fp8 (structure is validated; falls back to exact per-class
BLAS if pi is not a 0/1 path matrix). The device runs only the heavy
contraction C3 (137 GFLOP total) as fp8 DoubleRow matmuls. The tiny rank-1
corrections C1/C2 (<0.2% of FLOPs) are applied on the host in fp32.

Sharding splits the CONTRACTION: core k takes d in [16k, 16k+16) of both
XP and Y for ALL rows/cols, computes the full [1024,1024] partial C3 over
its d-slice, and the host sums the 8 fp32 partials. That makes both
operands SBUF-resident (8.4MB each; 20.8MB total DMA/core vs a ~116us PE
floor), so the kernel is cleanly PE-bound. To keep the full 128-partition
contraction with only 16 d's, the host packs 8 consecutive p's x 16 d's
per partition: part = 16*(p%8) + d_local, q = p//8. DoubleRow pairs run
over (q, q+1), i.e. K = 16 p's x 16 d's = 256 per matmul.

Device layouts (host pre-packs; every DMA contiguous):
  xq [part, q, i]  fp8, packed XP  (64KB/partition, resident)
  yq [part, q, j]  fp8, packed Y   (64KB/partition, resident)
Loop: 2 col-halves x 32 q-pairs x 8 row-tiles of DR matmuls (512 total,
N=512), accumulating 8 PSUM banks per col-half; rhs is shared across the
8 row-tiles of each (c, q-pair). Output c3 [1024, 1024] bf16 per core
(abs-error budget is ~35x larger than the bf16 rounding it adds), drained
to DRAM staggered so only the last tile's copy+DMA trails the final
matmul. A scratch-matmul burst at engine boot warms the PE HAM clock-gate
to 8/8 while the first operand chunks are still in flight.
"""

import os
import sys
import types

import numpy as np
import ml_dtypes

NX, NY, T, TP, D, C = 1024, 1024, 512, 512, 128, 8
N_CORES = 8
DL = D // N_CORES           # 16 d's per core
PP = 128 // DL              # 8 p's packed per partition column
NQ = TP // PP               # 64 q's
F8 = ml_dtypes.float8_e4m3fn


def _ensure_axon_hooks():
    """concourse.bass_utils imports antenv.axon_hooks when tracing under
    axon; some images lack that submodule. Provide it, and register the
    NTFF profile hook if the boot path didn't."""
    try:
        import antenv
    except ImportError:
        return
    try:
        from antenv import axon_hooks  # noqa: F401
    except ImportError:
        mod = types.ModuleType("antenv.axon_hooks")
        mod._hook = None

        def _set(h):
            mod._hook = h

        def _get():
            return mod._hook

        mod.set_axon_ntff_profile_hook = _set
        mod.get_axon_ntff_profile_hook = _get
        sys.modules["antenv.axon_hooks"] = mod
        antenv.axon_hooks = mod
    from antenv.axon_hooks import (
        get_axon_ntff_profile_hook,
        set_axon_ntff_profile_hook,
    )

    if get_axon_ntff_profile_hook() is None:
        try:
            from trn_agent_boot.trn_boot import _ntff_profile_via_ctypes

            hook = _ntff_profile_via_ctypes("/opt/axon/libaxon_pjrt.so")
            if hook is not None:
                set_axon_ntff_profile_hook(hook)
        except Exception:
            pass


_ensure_axon_hooks()

import concourse.bass as bass  # noqa: E402, F401
import concourse.tile as tile  # noqa: E402
from concourse import bacc, mybir  # noqa: E402
from concourse.bass_utils import run_bass_kernel_spmd  # noqa: E402

_PROGRAM_CACHE = {}
LAST_RUN = None  # BassKernelResults of the most recent kernel() call


def _build_program():
    if "nc" in _PROGRAM_CACHE:
        return _PROGRAM_CACHE["nc"]
    f8 = mybir.dt.float8e4
    f32 = mybir.dt.float32
    DR = mybir.MatmulPerfMode.DoubleRow
    nc = bacc.Bacc("TRN2", target_bir_lowering=False, debug=False,
                   num_devices=N_CORES)
    bf16 = mybir.dt.bfloat16
    xq_d = nc.dram_tensor("xq", [128, NQ, NX], f8, kind="ExternalInput").ap()
    yq_d = nc.dram_tensor("yq", [128, NQ, NY], f8, kind="ExternalInput").ap()
    c3 = nc.dram_tensor("c3", [NX, NY], bf16, kind="ExternalOutput").ap()

    with tile.TileContext(nc) as tc:
        with (
            tc.tile_pool(name="ops", bufs=1) as op_pool,
            tc.tile_pool(name="outsb", bufs=4) as out_pool,
        ):
            xq = op_pool.tile([128, NQ, NX], f8)
            yq = op_pool.tile([128, NQ, NY], f8)

            # PE warmup: ~3.4us of scratch matmuls starting at engine boot
            # (~7.3us), i.e. while no operand data exists yet, flip the HAM
            # clock-gate to 8/8 right as the first chunks land, so all real
            # matmuls run warm (values never read).
            wsrc = op_pool.tile([128, 128], f8)
            wdst = op_pool.tile([1, 8], f32)
            nc.vector.memset(wsrc[:], 0.0)
            with tc.tile_pool(name="warmps", bufs=1,
                              space="PSUM") as warmps_pool:
                wacc = warmps_pool.tile([128, 128], f32)
                for w in range(32):
                    nc.tensor.matmul(wacc[:], wsrc[:], wsrc[:],
                                     start=True, stop=True)

            # Operand loads in q-pair chunks aligned to s-blocks: matmul
            # block s waits on exactly one xq chunk + one yq chunk, and
            # DMA supply (~0.40 MB/us) stays ahead of warm consumption
            # (~0.29 MB/us) from the first block on.
            QC = 2
            for g in range(NQ // QC):
                a, b = g * QC, (g + 1) * QC
                nc.sync.dma_start(xq[:, a:b, :], xq_d[:, a:b, :])
                nc.sync.dma_start(yq[:, a:b, :], yq_d[:, a:b, :])
            # One tiny ACT op so the lazy activation-table load (~1.3us)
            # runs at boot, long before the first PSUM drain needs the
            # Scalar engine at ~62us.
            nc.scalar.copy(wdst[:], wsrc[0:1, 0:8])

            # Partial C3 over this core's d-slice: 2 col-halves x 32 q-pairs
            # x 8 row-tiles; 8 PSUM banks accumulate one col-half's row
            # tiles, then drain to SBUF/DRAM while the other half runs.
            # Col-half 0 iterates q-outer (chunk-streaming friendly);
            # col-half 1 (operands fully resident by then) iterates
            # row-tile-outer so the 8 tiles finish staggered and their
            # copies + output DMAs overlap compute instead of draining
            # serially after the last matmul.
            with tc.tile_pool(name="psB", bufs=1, space="PSUM") as psB_pool:
                def drain(rt, c, acc, split=False):
                    out = out_pool.tile([128, 512], bf16, name="out",
                                        tag="out")
                    h = 256
                    nc.vector.tensor_copy(out[:, 0:h], acc[:, 0:h])
                    nc.scalar.copy(out[:, h:512], acc[:, h:512])
                    dst = c3[rt * 128:(rt + 1) * 128,
                             c * 512:(c + 1) * 512]
                    if split:
                        # Last tile: two half-DMAs so each half departs as
                        # its copy lands and the ~2us completion receipts
                        # overlap instead of chaining.
                        nc.sync.dma_start(dst[:, 0:h], out[:, 0:h])
                        nc.sync.dma_start(dst[:, h:512], out[:, h:512])
                    else:
                        nc.sync.dma_start(dst, out[:])

                accs = [psB_pool.tile([128, 512], f32, name=f"acc{rt}",
                                      tag=f"acc{rt}")
                        for rt in range(8)]
                for s in range(NQ // 2):
                    q = 2 * s
                    st, sp = (s == 0), (s == NQ // 2 - 1)
                    rhs = yq[:, q:q + 2, 0:512]
                    for rt in range(8):
                        nc.tensor.matmul(
                            accs[rt][:],
                            xq[:, q:q + 2, rt * 128:(rt + 1) * 128],
                            rhs, start=st, stop=sp, perf_mode=DR)
                for rt in range(8):
                    drain(rt, 0, accs[rt])

                for rt in range(8):
                    acc = psB_pool.tile([128, 512], f32, name=f"acc{rt}",
                                        tag=f"acc{rt}")
                    for s in range(NQ // 2):
                        q = 2 * s
                        st, sp = (s == 0), (s == NQ // 2 - 1)
                        nc.tensor.matmul(
                            acc[:], xq[:, q:q + 2, rt * 128:(rt + 1) * 128],
                            yq[:, q:q + 2, 512:1024],
                            start=st, stop=sp, perf_mode=DR)
                    drain(rt, 1, acc, split=(rt == 7))

    nc.compile()
    _PROGRAM_CACHE["nc"] = nc
    return nc


def _host_xp(X, pi, classe):
    """XP[i, p, d] = sum_t pi[cl(i), t, p] * X[i, t, d], exact in fp32.

    Fast path uses the DTW-path structure (each pi column = contiguous run
    of ones): XP rows are differences of two prefix sums of X over t.
    Falls back to per-class BLAS if pi is not a 0/1 path matrix.
    """
    lo = np.empty((C, TP), np.int64)
    hi = np.empty((C, TP), np.int64)
    pib = pi != 0
    ok = bool(np.isin(pi, (0.0, 1.0)).all())
    if ok:
        for c in range(C):
            m = pib[c]
            cnt = m.sum(0)
            lo_c = m.argmax(0)
            hi_c = T - 1 - m[::-1].argmax(0)
            if not ((cnt > 0).all() and ((hi_c - lo_c + 1) == cnt).all()):
                ok = False
                break
            lo[c], hi[c] = lo_c, hi_c
    if ok:
        Xcp = np.zeros((NX, T + 1, D), np.float32)
        np.cumsum(X, axis=1, out=Xcp[:, 1:])
        ar = np.arange(NX)[:, None]
        XP = Xcp[ar, hi[classe] + 1] - Xcp[ar, lo[classe]]
    else:
        XP = np.empty((NX, TP, D), np.float32)
        for c in range(C):
            rows = np.nonzero(classe == c)[0]
            if rows.size:
                XP[rows] = np.einsum("ntd,tp->npd", X[rows], pi[c],
                                     optimize=True)
    return XP


def _pack(A8):
    """[n, TP, D] fp8 -> T[pp, d, q, n] contiguous, so core k's operand is
    T[:, DL*k:DL*(k+1)].reshape(128, NQ, n) with part = PP*(p%PP)+d_local
    ... i.e. part = DL*(p%PP) + d_local, q = p//PP."""
    n = A8.shape[0]
    T_ = A8.reshape(n, NQ, PP, D).transpose(2, 3, 1, 0)   # [pp, d, q, n]
    return np.ascontiguousarray(T_)


def kernel(X, Y, pi, classe):
    global LAST_RUN
    assert X.shape == (NX, T, D) and Y.shape == (NY, TP, D)
    assert pi.shape == (C, T, TP) and classe.shape == (NX,)
    X = np.asarray(X, dtype=np.float32)
    Y = np.asarray(Y, dtype=np.float32)
    pi = np.asarray(pi, dtype=np.float32)
    classe = np.asarray(classe)

    nc = _build_program()

    XP = _host_xp(X, pi, classe)                       # [NX, TP, D] fp32
    xq_t = _pack(XP.astype(F8))                        # [PP, D, NQ, NX]
    yq_t = _pack(Y.astype(F8))                         # [PP, D, NQ, NY]
    in_maps = []
    for k in range(N_CORES):
        ds = slice(DL * k, DL * (k + 1))
        in_maps.append({
            "xq": np.ascontiguousarray(xq_t[:, ds]).reshape(128, NQ, NX),
            "yq": np.ascontiguousarray(yq_t[:, ds]).reshape(128, NQ, NY),
        })

    trace = bool(os.environ.get("BASS_TRACE"))
    LAST_RUN = run_bass_kernel_spmd(nc, in_maps, list(range(N_CORES)),
                                    trace=trace)
    C3 = LAST_RUN.results[0]["c3"].astype(np.float32)
    for k in range(1, N_CORES):
        C3 += LAST_RUN.results[k]["c3"].astype(np.float32)

    # Host epilogue: rank-1 corrections (0.15% of FLOPs).
    row_c = pi.sum(-1)                                 # [C, T]
    col_c = pi.sum(1)                                  # [C, TP]
    SX = np.einsum("itd,itd->it", X, X)                # [NX, T]
    SY = np.einsum("jpd,jpd->jp", Y, Y)                # [NY, TP]
    C1 = np.einsum("it,it->i", SX, row_c[classe])      # [NX]
    C2 = col_c @ SY.T                                  # [C, NY]
    return (C1[:, None] + C2[classe] - 2.0 * C3).astype(np.float32)
